# revision 1
# baseline (speedup 1.0000x reference)
"""CNOT permutation kernel for Trainium2 (8 NeuronCores).

The reference op is ``out[lin[i]] = x[i]`` where ``lin`` is the CNOT
permutation on d^n basis states (d=2, n=24, control=0, target=1, batch=4).
``lin`` only edits the *target* digit of the row index, so over any row
range where the control/target digits are constant it is ``i + const``:
the whole permutation is a swap of contiguous row blocks.  We therefore
shard the 2^24 rows into 8 contiguous chunks (one per core), hand core c
the *source* block for its destination chunk, and each core runs a pure
DRAM->DRAM DMA memcpy of its 32 MiB shard — the memory-roofline minimum
traffic (read each input byte once, write each output byte once).
"""

import numpy as np

import concourse.bass as bass
import concourse.mybir as mybir
from concourse.bass_utils import run_bass_kernel_spmd

N_CORES = 8
ROWS = 1 << 24  # d ** n
BATCH = 4
CHUNK = ROWS // N_CORES

_NC = None


def _get_nc():
    """Build (once) the per-core Bass program: one 32 MiB DRAM->DRAM copy."""
    global _NC
    if _NC is None:
        nc = bass.Bass(trn_type="TRN2")
        x = nc.dram_tensor("x", [CHUNK, BATCH], mybir.dt.float32, kind="ExternalInput")
        y = nc.dram_tensor("y", [CHUNK, BATCH], mybir.dt.float32, kind="ExternalOutput")
        with nc.Block() as block, nc.semaphore("dma_sem") as dma_sem:

            @block.sync
            def _(sync):
                sync.dma_start(out=y[:], in_=x[:]).then_inc(dma_sem, 16)
                sync.wait_ge(dma_sem, 16)

        _NC = nc
    return _NC


def _src_starts(control, target, d, n):
    """Start row in x of the source block feeding each destination chunk.

    Valid when every chunk has constant control/target digits, i.e. the
    digit place values are multiples of CHUNK.  Returns None otherwise.
    """
    Dn = d**n
    if Dn != ROWS or control == target:
        return None
    pt = d ** (n - 1 - target)
    pc = d ** (n - 1 - control)
    if pt % CHUNK or pc % CHUNK:
        return None
    starts = []
    for c in range(N_CORES):
        j0 = c * CHUNK
        dt = (j0 // pt) % d
        dc = (j0 // pc) % d
        # out[j] = x[j + (((dt - dc) % d) - dt) * pt]  (inverse of lin)
        starts.append(j0 + (((dt - dc) % d) - dt) * pt)
    return starts


def _shards(x, control, target, d, n):
    """Per-core source shards of x (views when block-aligned)."""
    starts = _src_starts(control, target, d, n)
    if starts is not None:
        return [x[s : s + CHUNK] for s in starts]
    # Generic fallback: materialize the inverse permutation on the host.
    Dn = d**n
    idx = np.arange(Dn, dtype=np.int64)
    pt = d ** (n - 1 - target)
    pc = d ** (n - 1 - control)
    dt = (idx // pt) % d
    dc = (idx // pc) % d
    lin = idx + (((dt + dc) % d) - dt) * pt
    inv = np.empty(Dn, dtype=np.int64)
    inv[lin] = idx
    return [x[inv[c * CHUNK : (c + 1) * CHUNK]] for c in range(N_CORES)]


def _run(shards, **kwargs):
    in_maps = [{"x": s} for s in shards]
    res = run_bass_kernel_spmd(
        _get_nc(), in_maps, core_ids=list(range(N_CORES)), **kwargs
    )
    out = np.concatenate([res.results[c]["y"] for c in range(N_CORES)], axis=0)
    return out, res


def kernel(x, control, target, d, n):
    x = np.asarray(x)
    control = int(np.asarray(control))
    target = int(np.asarray(target))
    d = int(np.asarray(d))
    n = int(np.asarray(n))
    assert x.shape == (ROWS, BATCH), x.shape
    out, _ = _run(_shards(x, control, target, d, n))
    return out
